# revision 12
# baseline (speedup 1.0000x reference)
"""Bass/Tile Trainium2 kernel: batched scaled-dot-product attention.

Problem: B=8, Q=S=2048, D=1024 fp32.
  out[b] = softmax(Q[b] @ K[b] / sqrt(D)) @ V[b]
  (keys arrive pre-transposed as [B, D, S])

Sharding: pure data-parallel — one batch element per NeuronCore, 8 cores,
no collectives.

Per-core algorithm (S^T layout, avoids transposing the 2048x2048 score
matrix):
  for each q-chunk (512 rows of Q):
    QT = Q-chunk transposed on the TensorEngine (PE transpose via identity)
    for each s-tile (128 keys):
      ST[s,qc] = sum_d K[d,s-tile]^T-matmul  (lhsT = K tile, rhs = QT)
      PT[s,qc] = exp(ST * 1/sqrt(D))        (ACT, PSUM -> SBUF, fp32r round)
    for each q-subtile (128 rows):
      O = sum_s PT^T @ V      + rowsum via ones-column matmul (N=1)
      out = O * (1/rowsum)    (DVE reciprocal + tensor_scalar_mul)

Numerics: softmax without max-subtraction is exact here (scores ~N(0,1);
exp never overflows in fp32). Matmuls run in bf16 (fp32 accumulate in PSUM;
separate LDWEIGHTS pipelines under the previous matmul, FWL 2x loads).
"""
import sys

sys.path.insert(0, "/opt/trn_rl_repo")

import numpy as np

import concourse.bass as bass
import concourse.tile as tile
from concourse import mybir
from concourse.bass_utils import run_bass_kernel_spmd
from concourse.masks import make_identity

F32 = mybir.dt.float32
F32R = mybir.dt.float32r
BF16 = mybir.dt.bfloat16

B, Q, S, D = 8, 2048, 2048, 1024
SCALE = 1.0 / float(np.sqrt(D))

QC = 512                # q-chunk width (rhs moving dim of the S^T matmul)
N_QC = Q // QC          # 4 chunks
N_ST = S // 128         # 16 s-tiles
N_DT = D // 128         # 8 d-tiles
N_QT = QC // 128        # 4 q-subtiles per chunk
N_NC = D // 512         # 2 output column chunks of 512

# ---------------------------------------------------------------------------
# walrus in this container accepts only ONE sync-wait command per
# instruction; Tile attaches several at phase boundaries.  Split the excess
# onto preceding (in-order, same-engine) NoOps.
_uid = [0]


def _mk_nop(engine, waits):
    _uid[0] += 1
    return mybir.InstNoOp(
        name=f"I-waitsplit-{_uid[0]}",
        engine=engine,
        ins=[],
        outs=[],
        bass_nofuse=True,
        sync_info=mybir.SyncInfo(on_wait=list(waits), on_update=[]),
    )


def split_excess_waits(nc, limit=1):
    for f in nc.m.functions:
        for bb in f.blocks:
            out = []
            for inst in bb.instructions:
                si = inst.sync_info
                waits = list(si.on_wait) if si and si.on_wait else []
                if len(waits) > limit:
                    keep = waits[-limit:]
                    excess = waits[: len(waits) - limit]
                    for i in range(0, len(excess), limit):
                        out.append(_mk_nop(inst.engine, excess[i : i + limit]))
                    si.on_wait = keep
                out.append(inst)
            bb.instructions[:] = out
    return nc


# ---------------------------------------------------------------------------
def build(reps: int = 1):
    nc = bass.Bass()
    q_ext = nc.declare_dram_parameter("queries", [Q, D], F32, isOutput=False)
    k_ext = nc.declare_dram_parameter("keys", [D, S], F32, isOutput=False)
    v_ext = nc.declare_dram_parameter("values", [S, D], F32, isOutput=False)
    o_ext = nc.declare_dram_parameter("out", [Q, D], F32, isOutput=True)

    with tile.TileContext(nc) as tc:
        with (
            tc.tile_pool(name="resident", bufs=1) as resident,
            tc.tile_pool(name="qstage", bufs=6) as qstage,
            tc.tile_pool(name="qt_pool", bufs=2) as qt_pool,
            tc.tile_pool(name="pt_pool", bufs=1) as pt_pool,
            tc.tile_pool(name="osb_pool", bufs=3) as osb_pool,
            tc.tile_pool(name="rs_pool", bufs=2) as rs_pool,
            tc.tile_pool(name="qk_ps", bufs=2, space="PSUM") as qk_ps,
            tc.tile_pool(name="pv_ps", bufs=4, space="PSUM") as pv_ps,
            tc.tile_pool(name="sum_ps", bufs=2, space="PSUM") as sum_ps,
        ):
            for _rep in range(reps):
                _build_one_pass(nc, tc, resident, qstage, qt_pool, pt_pool,
                                osb_pool, rs_pool, qk_ps, pv_ps, sum_ps,
                                q_ext, k_ext, v_ext, o_ext)

    split_excess_waits(nc)
    return nc


def _build_one_pass(nc, tc, resident, qstage, qt_pool, pt_pool, osb_pool,
                    rs_pool, qk_ps, pv_ps, sum_ps, q_ext, k_ext, v_ext, o_ext):
    if True:
        if True:
            # --- small constants + chunk-0 Q first, so the PE starts
            # transposing while K/V stream in -----------------------------
            ident = resident.tile([128, 128], F32, tag="ident")
            make_identity(nc, ident[:])
            ones = resident.tile([128, 2], BF16, tag="ones")
            nc.gpsimd.memset(ones[:], 1.0)

            def load_and_transpose_chunk(c):
                """DMA Q rows for chunk c and produce QT (f32r) via PE."""
                stage = []
                for i in range(N_QT):
                    qs = qstage.tile([128, D], F32, tag="qs")
                    nc.sync.dma_start(
                        qs[:], q_ext[c * QC + i * 128 : c * QC + (i + 1) * 128, :]
                    )
                    stage.append(qs)
                qt_c = qt_pool.tile([128, N_DT * QC], BF16, tag="qt")
                for d in range(N_DT):
                    pst = qk_ps.tile([128, QC], F32, tag="qkps")
                    for i in range(N_QT):
                        nc.tensor.transpose(
                            pst[:, i * 128 : (i + 1) * 128],
                            stage[i][:, d * 128 : (d + 1) * 128],
                            ident[:],
                        )
                    nc.any.tensor_copy(qt_c[:, d * QC : (d + 1) * QC], pst[:])
                return qt_c

            qt = load_and_transpose_chunk(0)

            # --- resident K/V: separate tiles per 128-row block so each
            # matmul only waits on its own DMA ---------------------------
            ksb = [
                resident.tile([128, S], BF16, name=f"ksb{d}", tag=f"ksb{d}")
                for d in range(N_DT)
            ]
            for d in range(N_DT):
                nc.gpsimd.dma_start(ksb[d][:], k_ext[d * 128 : (d + 1) * 128, :])
            vsb = [
                resident.tile([128, D], BF16, name=f"vsb{s}", tag=f"vsb{s}")
                for s in range(N_ST)
            ]
            for s in range(N_ST):
                nc.gpsimd.dma_start(vsb[s][:], v_ext[s * 128 : (s + 1) * 128, :])

            for c in range(N_QC):
                # --- S^T = K^T-style matmul; exp into PT ----------------
                pt = [
                    pt_pool.tile([128, QC], BF16, name=f"pt{s}", tag=f"pt{s}")
                    for s in range(N_ST)
                ]
                for s in range(N_ST):
                    ps = qk_ps.tile([128, QC], F32, tag="qkps")
                    for d in range(N_DT):
                        nc.tensor.matmul(
                            ps[:],
                            ksb[d][:, s * 128 : (s + 1) * 128],
                            qt[:, d * QC : (d + 1) * QC],
                            start=(d == 0),
                            stop=(d == N_DT - 1),
                        )
                    nc.scalar.activation(
                        pt[s][:],
                        ps[:],
                        mybir.ActivationFunctionType.Exp,
                        scale=SCALE,
                    )

                # --- prefetch + transpose next chunk's Q (fills the gap
                # while the last exp of this chunk drains) ---------------
                qt_next = None
                if c + 1 < N_QC:
                    qt_next = load_and_transpose_chunk(c + 1)

                # --- O = PT^T @ [V, 1] ; normalize ----------------------
                for i in range(N_QT):
                    pv = [
                        pv_ps.tile([128, 512], F32, name=f"pv{n}", tag="pvps")
                        for n in range(N_NC)
                    ]
                    rs = sum_ps.tile([128, 2], F32, tag="sumps")
                    for s in range(N_ST):
                        lhs = pt[s][:, i * 128 : (i + 1) * 128]
                        st, sp = (s == 0), (s == N_ST - 1)
                        for n in range(N_NC):
                            nc.tensor.matmul(
                                pv[n][:],
                                lhs,
                                vsb[s][:, n * 512 : (n + 1) * 512],
                                start=st,
                                stop=sp,
                            )
                        nc.tensor.matmul(rs[:], lhs, ones[:, 0:2], start=st, stop=sp)
                    rinv = rs_pool.tile([128, 1], F32, tag="rinv")
                    nc.vector.reciprocal(rinv[:], rs[:, 0:1])
                    osb = osb_pool.tile([128, D], F32, tag="osb")
                    for n in range(N_NC):
                        nc.vector.tensor_scalar_mul(
                            osb[:, n * 512 : (n + 1) * 512], pv[n][:], rinv[:]
                        )
                    nc.sync.dma_start(
                        o_ext[c * QC + i * 128 : c * QC + (i + 1) * 128, :], osb[:]
                    )

                if qt_next is not None:
                    qt = qt_next


_CACHED = {}


def kernel(queries: np.ndarray, keys: np.ndarray, values: np.ndarray) -> np.ndarray:
    assert queries.shape == (B, Q, D)
    assert keys.shape == (B, D, S)
    assert values.shape == (B, S, D)
    if "nc" not in _CACHED:
        _CACHED["nc"] = build()
    nc = _CACHED["nc"]
    in_maps = [
        {
            "queries": np.ascontiguousarray(queries[b], dtype=np.float32),
            "keys": np.ascontiguousarray(keys[b], dtype=np.float32),
            "values": np.ascontiguousarray(values[b], dtype=np.float32),
        }
        for b in range(B)
    ]
    res = run_bass_kernel_spmd(nc, in_maps, core_ids=list(range(B)))
    out = np.stack([res.results[b]["out"] for b in range(B)], axis=0)
    return out.astype(np.float32)


if __name__ == "__main__":
    rng = np.random.default_rng(0)
    q = rng.standard_normal((B, Q, D), dtype=np.float32)
    k = rng.standard_normal((B, D, S), dtype=np.float32)
    v = rng.standard_normal((B, S, D), dtype=np.float32)
    o = kernel(queries=q, keys=k, values=v)
    print("out", o.shape, o.dtype, np.abs(o).mean())


# revision 15
# speedup vs baseline: 2.3237x; 2.3237x over previous
"""Bass/Tile Trainium2 kernel: batched scaled-dot-product attention.

Problem: B=8, Q=S=2048, D=1024 fp32.
  out[b] = softmax(Q[b] @ K[b] / sqrt(D)) @ V[b]
  (keys arrive pre-transposed as [B, D, S])

Sharding: pure data-parallel — one batch element per NeuronCore, 8 cores,
no collectives.

Per-core algorithm (S^T layout, avoids transposing the 2048x2048 score
matrix):
  for each q-chunk (512 rows of Q):
    QT = Q-chunk transposed on the TensorEngine (via identity matmul)
    for each s-tile (128 keys):
      ST[s,qc] = sum_d K[d,s-tile]^T-matmul  (lhsT = K tile, rhs = QT)
      PT[s,qc] = exp(ST * 1/sqrt(D))        (ACT, PSUM -> SBUF, bf16)
    for each q-subtile (128 rows):
      O = sum_s PT^T @ V      + rowsum via ones-column matmul (N=1)
      out = O * (1/rowsum)    (DVE reciprocal + tensor_scalar_mul)

Numerics: softmax without max-subtraction is exact here (scores ~N(0,1);
exp never overflows in fp32). Matmuls run in bf16 (fp32 accumulate in PSUM;
separate LDWEIGHTS pipelines under the previous matmul, FWL 2x loads).
"""
import sys

sys.path.insert(0, "/opt/trn_rl_repo")

import numpy as np

import concourse.bass as bass
import concourse.tile as tile
from concourse import mybir
from concourse.bass_utils import run_bass_kernel_spmd
from concourse.masks import make_identity

F32 = mybir.dt.float32
F32R = mybir.dt.float32r
BF16 = mybir.dt.bfloat16

B, Q, S, D = 8, 2048, 2048, 1024
SCALE = 1.0 / float(np.sqrt(D))

QC = 512                # q-chunk width (rhs moving dim of the S^T matmul)
N_QC = Q // QC          # 4 chunks
N_ST = S // 128         # 16 s-tiles
N_DT = D // 128         # 8 d-tiles
N_QT = QC // 128        # 4 q-subtiles per chunk
N_NC = D // 512         # 2 output column chunks of 512

# ---------------------------------------------------------------------------
# walrus in this container accepts only ONE sync-wait command per
# instruction; Tile attaches several at phase boundaries.  Split the excess
# onto preceding (in-order, same-engine) NoOps.
_uid = [0]


def _mk_nop(engine, waits):
    _uid[0] += 1
    return mybir.InstNoOp(
        name=f"I-waitsplit-{_uid[0]}",
        engine=engine,
        ins=[],
        outs=[],
        bass_nofuse=True,
        sync_info=mybir.SyncInfo(on_wait=list(waits), on_update=[]),
    )


def split_excess_waits(nc, limit=1):
    for f in nc.m.functions:
        for bb in f.blocks:
            out = []
            for inst in bb.instructions:
                si = inst.sync_info
                waits = list(si.on_wait) if si and si.on_wait else []
                if len(waits) > limit:
                    keep = waits[-limit:]
                    excess = waits[: len(waits) - limit]
                    for i in range(0, len(excess), limit):
                        out.append(_mk_nop(inst.engine, excess[i : i + limit]))
                    si.on_wait = keep
                out.append(inst)
            bb.instructions[:] = out
    return nc


# ---------------------------------------------------------------------------
def build(reps: int = 1):
    nc = bass.Bass()
    q_ext = nc.declare_dram_parameter("queries", [Q, D], F32, isOutput=False)
    k_ext = nc.declare_dram_parameter("keys", [D, S], F32, isOutput=False)
    v_ext = nc.declare_dram_parameter("values", [S, D], F32, isOutput=False)
    o_ext = nc.declare_dram_parameter("out", [Q, D], F32, isOutput=True)

    with tile.TileContext(nc) as tc:
        with (
            tc.tile_pool(name="resident", bufs=1) as resident,
            tc.tile_pool(name="qstage", bufs=6) as qstage,
            tc.tile_pool(name="qt_pool", bufs=2) as qt_pool,
            tc.tile_pool(name="pt_pool", bufs=1) as pt_pool,
            tc.tile_pool(name="osb_pool", bufs=3) as osb_pool,
            tc.tile_pool(name="rs_pool", bufs=2) as rs_pool,
            tc.tile_pool(name="qk_ps", bufs=2, space="PSUM") as qk_ps,
            tc.tile_pool(name="pv_ps", bufs=4, space="PSUM") as pv_ps,
            tc.tile_pool(name="sum_ps", bufs=2, space="PSUM") as sum_ps,
        ):
            for _rep in range(reps):
                _build_one_pass(nc, tc, resident, qstage, qt_pool, pt_pool,
                                osb_pool, rs_pool, qk_ps, pv_ps, sum_ps,
                                q_ext, k_ext, v_ext, o_ext)

    split_excess_waits(nc)
    return nc


def _build_one_pass(nc, tc, resident, qstage, qt_pool, pt_pool, osb_pool,
                    rs_pool, qk_ps, pv_ps, sum_ps, q_ext, k_ext, v_ext, o_ext):
    if True:
        if True:
            # --- small constants + chunk-0 Q first, so the PE starts
            # transposing while K/V stream in -----------------------------
            ident = resident.tile([128, 128], F32, tag="ident")
            make_identity(nc, ident[:])
            ones = resident.tile([128, 2], BF16, tag="ones")
            nc.gpsimd.memset(ones[:], 1.0)

            def load_and_transpose_chunk(c):
                """DMA Q rows for chunk c; transpose on the PE via identity."""
                stage = []
                for i in range(N_QT):
                    qs = qstage.tile([128, D], F32, tag="qs")
                    nc.sync.dma_start(
                        qs[:], q_ext[c * QC + i * 128 : c * QC + (i + 1) * 128, :]
                    )
                    stage.append(qs)
                qt_c = qt_pool.tile([128, N_DT * QC], BF16, tag="qt")
                for d in range(N_DT):
                    pst = qk_ps.tile([128, QC], F32, tag="qkps")
                    for i in range(N_QT):
                        nc.tensor.transpose(
                            pst[:, i * 128 : (i + 1) * 128],
                            stage[i][:, d * 128 : (d + 1) * 128],
                            ident[:],
                        )
                    nc.any.tensor_copy(qt_c[:, d * QC : (d + 1) * QC], pst[:])
                return qt_c

            qt = load_and_transpose_chunk(0)

            # --- resident K/V: separate tiles per 128-row block so each
            # matmul only waits on its own DMA ---------------------------
            ksb = [
                resident.tile([128, S], BF16, name=f"ksb{d}", tag=f"ksb{d}")
                for d in range(N_DT)
            ]
            for d in range(N_DT):
                nc.gpsimd.dma_start(ksb[d][:], k_ext[d * 128 : (d + 1) * 128, :])
            vsb = [
                resident.tile([128, D], BF16, name=f"vsb{s}", tag=f"vsb{s}")
                for s in range(N_ST)
            ]
            for s in range(N_ST):
                nc.gpsimd.dma_start(vsb[s][:], v_ext[s * 128 : (s + 1) * 128, :])

            for c in range(N_QC):
                # --- S^T = K^T-style matmul; exp into PT ----------------
                pt = [
                    pt_pool.tile([128, QC], BF16, name=f"pt{s}", tag=f"pt{s}")
                    for s in range(N_ST)
                ]
                for s in range(N_ST):
                    ps = qk_ps.tile([128, QC], F32, tag="qkps")
                    for d in range(N_DT):
                        nc.tensor.matmul(
                            ps[:],
                            ksb[d][:, s * 128 : (s + 1) * 128],
                            qt[:, d * QC : (d + 1) * QC],
                            start=(d == 0),
                            stop=(d == N_DT - 1),
                        )
                    nc.scalar.activation(
                        pt[s][:],
                        ps[:],
                        mybir.ActivationFunctionType.Exp,
                        scale=SCALE,
                    )

                # --- prefetch + transpose next chunk's Q (fills the gap
                # while the last exp of this chunk drains) ---------------
                qt_next = None
                if c + 1 < N_QC:
                    qt_next = load_and_transpose_chunk(c + 1)

                # --- O = PT^T @ [V, 1] ; normalize ----------------------
                for i in range(N_QT):
                    pv = [
                        pv_ps.tile([128, 512], F32, name=f"pv{n}", tag="pvps")
                        for n in range(N_NC)
                    ]
                    rs = sum_ps.tile([128, 2], F32, tag="sumps")
                    for s in range(N_ST):
                        lhs = pt[s][:, i * 128 : (i + 1) * 128]
                        st, sp = (s == 0), (s == N_ST - 1)
                        for n in range(N_NC):
                            nc.tensor.matmul(
                                pv[n][:],
                                lhs,
                                vsb[s][:, n * 512 : (n + 1) * 512],
                                start=st,
                                stop=sp,
                            )
                        nc.tensor.matmul(rs[:], lhs, ones[:, 0:2], start=st, stop=sp)
                    rinv = rs_pool.tile([128, 1], F32, tag="rinv")
                    nc.vector.reciprocal(rinv[:], rs[:, 0:1])
                    osb = osb_pool.tile([128, D], F32, tag="osb")
                    for n in range(N_NC):
                        nc.vector.tensor_scalar_mul(
                            osb[:, n * 512 : (n + 1) * 512], pv[n][:], rinv[:]
                        )
                    nc.sync.dma_start(
                        o_ext[c * QC + i * 128 : c * QC + (i + 1) * 128, :], osb[:]
                    )

                if qt_next is not None:
                    qt = qt_next


_CACHED = {}


def kernel(queries: np.ndarray, keys: np.ndarray, values: np.ndarray) -> np.ndarray:
    assert queries.shape == (B, Q, D)
    assert keys.shape == (B, D, S)
    assert values.shape == (B, S, D)
    if "nc" not in _CACHED:
        _CACHED["nc"] = build()
    nc = _CACHED["nc"]
    in_maps = [
        {
            "queries": np.ascontiguousarray(queries[b], dtype=np.float32),
            "keys": np.ascontiguousarray(keys[b], dtype=np.float32),
            "values": np.ascontiguousarray(values[b], dtype=np.float32),
        }
        for b in range(B)
    ]
    res = run_bass_kernel_spmd(nc, in_maps, core_ids=list(range(B)))
    out = np.stack([res.results[b]["out"] for b in range(B)], axis=0)
    return out.astype(np.float32)


if __name__ == "__main__":
    rng = np.random.default_rng(0)
    q = rng.standard_normal((B, Q, D), dtype=np.float32)
    k = rng.standard_normal((B, D, S), dtype=np.float32)
    v = rng.standard_normal((B, S, D), dtype=np.float32)
    o = kernel(queries=q, keys=k, values=v)
    print("out", o.shape, o.dtype, np.abs(o).mean())
